# revision 18
# baseline (speedup 1.0000x reference)
"""BitLinear (ternary absmean-quantized linear) on 8 TRN2 NeuronCores.

Reference math (fp32):
    gamma = mean(|W|)
    Wq    = round(clip(W / (gamma + 1e-5), -1, 1))   # ternary {-1, 0, 1}
    out   = einsum('bsi,oi->bso', x, Wq)             # x @ Wq.T

Sharding: data-parallel over tokens. x [4,2048,4096] -> 8192 tokens, each
core owns 1024 of them and computes its full [1024, 4096] output slab with
no output collective. Every core needs the full quantized W; gamma (a global
scalar) is computed cooperatively: each core abs-sums 1/8 of W (512 of the
4096 output rows), a tiny [128,1] AllReduce combines the partials, and each
core then quantizes the full W on the fly while the TensorEngine consumes it.

Precision: hybrid bf16 / fp8-DoubleRow along K. Ternary weights are exact in
both bf16 and e4m3. x is exact enough in bf16 (rel err ~2e-3) and marginal in
e4m3 alone (~2.7e-2 > the 2e-2 gate), so the first KB K-planes run as bf16
matmuls and the remaining K-KB planes run as fp8 e4m3 matmuls with
perf_mode=DoubleRow (2 fp8 weights per PE cell, ~1.5-2x bf16 throughput).
Total rel err ~ 2.66e-2 * sqrt((K-KB)/K) ~= 1.8e-2 at KB=2176.

Device kernel layout (per core), output TRANSPOSED (features x tokens):
    xT   [4096, 1024] bf16  - this core's x slab, K-major (moving operand)
    WT   [4096, 4096] f32   - full W, transposed (in_features major)
    Wg   [4096,  512] f32   - this core's gamma shard (= 512 columns of WT)
    outT [4096, 1024] f32   - out.T; host transposes back

Main loop: 8 N-chunks of 512 output features. Per chunk: stream 32 K-slabs
of WT, quantize each on DVE ((|w| > t) with sign, 2 ops) into a resident
bf16 [128, KTB, 512] + fp8 [128, KTF, 512] chunk. Matmuls run with the
quantized W slab as the STATIONARY operand (one LDWEIGHTS per 2 matmuls of
512 tokens) and x as the moving operand, so weight loads amortize and the
PE streams 512-wide: per n-tile of 128 features, KTB bf16 matmuls + KTF/2
DoubleRow matmuls accumulate the full-K dot product in one PSUM bank.
"""

import numpy as np
import ml_dtypes

NCORES = 8

# Full-problem dims (hardcoded per the harness contract).
B, S, D_IN, D_OUT = 4, 2048, 4096, 4096
M_TOTAL = B * S            # 8192 tokens
N_SPLIT = 2                # n-halves (1: 8 token groups; 2: 4 groups x 2)
M_GROUPS = NCORES // N_SPLIT
M_CORE = M_TOTAL // M_GROUPS   # tokens per core
N_CORE = D_OUT // N_SPLIT      # output features per core
KB_PLANES = 2048           # K-planes computed in bf16 (rest fp8 DoubleRow)

_COMPILED = None   # cached (nc, meta)
LAST_RESULTS = None  # BassKernelResults of the most recent run (for test.py)


def build_module(m_core=M_CORE, k=D_IN, n=N_CORE, ncores=NCORES, repeat=1,
                 use_collective=True, kb=KB_PLANES, n_full=None,
                 wpool_bufs=None, stg_bufs=None, g_chunk=None):
    if wpool_bufs is None:
        wpool_bufs = 12 if m_core <= 1024 else 6
    if stg_bufs is None:
        stg_bufs = 4 if m_core <= 1024 else 2
    if g_chunk is None:
        g_chunk = 4 if m_core <= 1024 else 2
    # SBUF squeeze for the large-m (4x2-sharded) config
    opool_bufs = 6 if m_core <= 1024 else 4
    spool_bufs = 6 if m_core <= 1024 else 4
    """Build + compile the SPMD Bass module. Parametrized so a shrunken
    config can be validated in CoreSim. repeat>1 unrolls the whole kernel
    body multiple times inside one NEFF (for steady-state timing).

    n is this core's output-feature count; n_full (defaults to n) is the
    FULL problem's out_features, used only for the gamma normalization
    (gamma = sum|W| / (k * n_full), reduced across cores)."""
    import concourse.bass as bass  # noqa: F401
    import concourse.mybir as mybir
    import concourse.tile as tile
    from concourse import bacc
    from concourse import bass_isa

    f32 = mybir.dt.float32
    bf16 = mybir.dt.bfloat16
    f8 = mybir.dt.float8e4
    KT = k // 128            # total k-tiles of 128
    KTB = kb // 128          # bf16 k-tiles
    KTF = KT - KTB           # fp8 k-tiles
    KPAIRS = KTF // 2        # fp8 DoubleRow pairs
    KODD = KTF - 2 * KPAIRS  # leftover unpaired fp8 k-tile (0 or 1)
    NCHUNK = min(512, n)     # output-feature chunk width
    NCHUNKS = n // NCHUNK
    NTILES = NCHUNK // 128   # n-tiles (stationary free dim) per chunk
    MW = min(512, m_core)    # moving-operand token width
    MH = m_core // MW        # moving tiles per k-slab
    if n_full is None:
        n_full = n
    NG = n_full // 8         # gamma shard width (columns of full WT)
    G_CHUNK = min(g_chunk, KT)  # k-tiles per gamma reduce chunk
    G_CHUNKS = KT // G_CHUNK
    N_ELEMS = float(k * n_full)

    nc = bacc.Bacc("TRN2", target_bir_lowering=False, debug=False,
                   num_devices=ncores)
    xT = nc.dram_tensor("xT", [k, m_core], bf16, kind="ExternalInput")
    WT = nc.dram_tensor("WT", [k, n], f32, kind="ExternalInput")
    Wg = nc.dram_tensor("Wg", [k, NG], f32, kind="ExternalInput")
    outT = nc.dram_tensor("outT", [n, m_core], f32, kind="ExternalOutput")

    ts = bass.ts
    DR = mybir.MatmulPerfMode.DoubleRow

    with tile.TileContext(nc) as tc:
        with (
            tc.tile_pool(name="xpool", bufs=1) as xpool,
            tc.tile_pool(name="xstg", bufs=stg_bufs) as xstg,
            tc.tile_pool(name="gpool", bufs=2) as gpool,
            tc.tile_pool(name="wqbp", bufs=2) as wqbp,
            tc.tile_pool(name="wqfp", bufs=2) as wqfp,
            tc.tile_pool(name="wpool", bufs=wpool_bufs) as wpool,
            tc.tile_pool(name="spool", bufs=spool_bufs) as spool,
            tc.tile_pool(name="opool", bufs=opool_bufs) as opool,
            tc.tile_pool(name="small", bufs=2) as small,
            tc.tile_pool(name="pmain", bufs=8, space="PSUM") as pmain,
            tc.tile_pool(name="dram", bufs=2, space="DRAM") as dram,
        ):
          with tc.tile_pool(name="cpool", bufs=1) as cpool:
            bias_p = cpool.tile([128, 1], f32, name="bias_p")
            nc.gpsimd.memset(bias_p[:], 0.5e-5)
            bias_n = cpool.tile([128, 1], f32, name="bias_n")
            nc.gpsimd.memset(bias_n[:], -0.5e-5)

          # ---- resident x: bf16 [128, KTB, m] + fp8 [128, KTF, m] ----
          # Loaded once per NEFF execution (shared across repeat iterations;
          # x does not change within one launch). fp8 planes stage through
          # bf16 and cast on DVE.
          xb = xpool.tile([128, KTB, m_core], bf16, name="xb")
          xf = None
          if KTF:
              xf = xpool.tile([128, KTF, m_core], f8, name="xf")
          xr = xT[:, :].rearrange("(t p) m -> p t m", p=128)
          for kt in range(KT):
              if kt < KTB:
                  nc.sync.dma_start(xb[:, kt, :], xr[:, kt, :])
              else:
                  stg = xstg.tile([128, m_core], bf16, tag="xstg")
                  nc.sync.dma_start(stg[:], xr[:, kt, :])
                  nc.vector.tensor_copy(xf[:, kt - KTB, :], stg[:])

          for _rep in range(repeat):
            # ---- gamma: local abs-sum over this core's shard ----
            # Entirely on ACT + gpsimd (with its DMAs issued from the ACT
            # sequencer): these queues are idle during the main loop, so in
            # the repeat/steady-state case iteration i+1's whole gamma chain
            # (including the AllReduce) overlaps iteration i's matmuls
            # instead of queuing behind i's DVE/sync FIFOs.
            acc = small.tile([128, G_CHUNKS], f32)
            for j in range(G_CHUNKS):
                gsl = gpool.tile([128, G_CHUNK, NG], f32, tag="gsl")
                src = Wg[j * G_CHUNK * 128:(j + 1) * G_CHUNK * 128, :]
                # rep 0: sync queue -> gamma DMAs get strict head priority.
                # reps >0: ACT queue -> next iteration's gamma prefetch runs
                # under the current iteration's matmuls (sync FIFO is busy).
                geng = nc.sync if _rep == 0 else nc.scalar
                geng.dma_start(gsl[:], src.rearrange("(t p) c -> p t c", p=128))
                gscr = gpool.tile([128, G_CHUNK, NG], bf16, tag="gscr")
                nc.scalar.activation(
                    gscr[:], gsl[:], mybir.ActivationFunctionType.Abs,
                    accum_out=acc[:, j:j + 1])
            gpart = small.tile([128, 1], f32)
            gscr2 = small.tile([128, G_CHUNKS], bf16)
            nc.scalar.activation(
                gscr2[:], acc[:], mybir.ActivationFunctionType.Abs,
                accum_out=gpart[:])

            # ---- tiny AllReduce of per-partition partials ----
            gsum = small.tile([128, 1], f32)
            if ncores > 1 and use_collective:
                cin = dram.tile([128, 1], f32)
                nc.scalar.dma_start(cin[:], gpart[:])
                cout = dram.tile([128, 1], f32, tag="cout", name=f"cout{_rep}")
                nc.gpsimd.collective_compute(
                    "AllReduce", mybir.AluOpType.add,
                    replica_groups=[list(range(ncores))],
                    ins=[cin[:].opt()], outs=[cout[:].opt()])
                nc.scalar.dma_start(gsum[:], cout[:])
            else:
                # timing/TimelineSim variant: no collective (gamma from the
                # local shard only -- numerically wrong, timing-equivalent)
                nc.scalar.copy(gsum[:], gpart[:])

            # sum across partitions, result broadcast to all partitions
            gtot = small.tile([128, 1], f32)
            nc.gpsimd.partition_all_reduce(
                gtot[:], gsum[:], channels=128, reduce_op=bass_isa.ReduceOp.add)

            # threshold t = 0.5 * (gamma + 1e-5)
            # Wq = (w > t) - (w < -t)  in {-1, 0, 1}
            tsb = small.tile([128, 1], f32)
            nc.scalar.activation(
                tsb[:], gtot[:], mybir.ActivationFunctionType.Identity,
                bias=bias_p[:], scale=0.5 / N_ELEMS)
            ntsb = small.tile([128, 1], f32)
            nc.scalar.activation(
                ntsb[:], gtot[:], mybir.ActivationFunctionType.Identity,
                bias=bias_n[:], scale=-0.5 / N_ELEMS)

            # ---- main loop over output-feature chunks ----
            for c in range(NCHUNKS):
                # quantize this chunk's W: stream 32 k-slabs, 2 DVE ops each
                wqb = wqbp.tile([128, KTB, NCHUNK], bf16, tag="wqb")
                wqf = None
                if KTF:
                    wqf = wqfp.tile([128, KTF, NCHUNK], f8, tag="wqf",
                                    name="wqf")
                for kt in range(KT):
                    wtmp = wpool.tile([128, NCHUNK], f32, tag="wtmp")
                    nc.sync.dma_start(
                        wtmp[:], WT[ts(kt, 128), ts(c, NCHUNK)])
                    neg = spool.tile([128, NCHUNK], bf16, tag="neg")
                    nc.vector.tensor_scalar(
                        neg[:], wtmp[:], ntsb[:], None, mybir.AluOpType.is_lt)
                    dst = (wqb[:, kt, :] if kt < KTB
                           else wqf[:, kt - KTB, :])
                    nc.vector.scalar_tensor_tensor(
                        dst, wtmp[:], tsb[:], neg[:],
                        mybir.AluOpType.is_gt, mybir.AluOpType.subtract)

                # matmuls: stationary = wq n-tile (128 features), moving = x
                # (MW tokens). One PSUM bank accumulates full K per (nt, mh).
                for nt in range(NTILES):
                    ps = [pmain.tile([128, MW], f32, tag="ps",
                                     name=f"ps{nt % 2}_{mh}")
                          for mh in range(MH)]
                    n0 = nt * 128
                    for kt in range(KTB):
                        lw = wqb[:, kt, n0:n0 + 128]
                        for mh in range(MH):
                            nc.tensor.matmul(
                                ps[mh][:], lw, xb[:, kt, ts(mh, MW)],
                                start=(kt == 0), stop=False)
                    for j in range(KPAIRS):
                        lw = wqf[:, 2 * j:2 * j + 2, n0:n0 + 128]
                        last = (j == KPAIRS - 1) and KODD == 0
                        for mh in range(MH):
                            nc.tensor.matmul(
                                ps[mh][:], lw,
                                xf[:, 2 * j:2 * j + 2, ts(mh, MW)],
                                start=False, stop=last, perf_mode=DR)
                    if KODD:
                        lw = wqf[:, KTF - 1, n0:n0 + 128]
                        for mh in range(MH):
                            nc.tensor.matmul(
                                ps[mh][:], lw, xf[:, KTF - 1, ts(mh, MW)],
                                start=False, stop=True)
                    for mh in range(MH):
                        osb = opool.tile([128, MW], f32, tag="osb")
                        # PSUM drain on ACT (near-idle): keeps bank release
                        # off the DVE quantization stream's in-order queue.
                        nc.scalar.copy(osb[:], ps[mh][:])
                        nc.sync.dma_start(
                            outT[c * NCHUNK + n0:c * NCHUNK + n0 + 128,
                                 ts(mh, MW)], osb[:])

    nc.compile()
    meta = dict(m_core=m_core, k=k, n=n, ncores=ncores, NG=NG, kb=kb)
    return nc, meta


def _get_compiled():
    global _COMPILED
    if _COMPILED is None:
        _COMPILED = build_module(n_full=D_OUT)
    return _COMPILED


def make_in_maps(x, W, m_core=M_CORE, n_core=N_CORE, ncores=NCORES,
                 n_split=N_SPLIT):
    """Host-side shard prep. x [B,S,D_IN] f32, W [D_OUT,D_IN] f32.
    Core c = (token-group c//n_split, n-half c%n_split)."""
    k = W.shape[1]
    n = W.shape[0]
    ng = n // ncores
    x2 = np.asarray(x, dtype=np.float32).reshape(-1, k)
    xb = x2.astype(ml_dtypes.bfloat16)
    WT = np.ascontiguousarray(np.asarray(W, dtype=np.float32).T)  # [k, n]
    xTg = [np.ascontiguousarray(xb[g * m_core:(g + 1) * m_core, :].T)
           for g in range(ncores // n_split)]
    in_maps = []
    for c in range(ncores):
        g, h = divmod(c, n_split)
        WTc = (WT if n_split == 1 else
               np.ascontiguousarray(WT[:, h * n_core:(h + 1) * n_core]))
        Wgc = np.ascontiguousarray(WT[:, c * ng:(c + 1) * ng])
        in_maps.append({"xT": xTg[g], "WT": WTc, "Wg": Wgc})
    return in_maps


def kernel(input, W):
    """Full inputs in, full output out. Shards internally across 8 cores."""
    global LAST_RESULTS
    from concourse import bass_utils

    nc, meta = _get_compiled()
    in_maps = make_in_maps(input, W)
    res = bass_utils.run_bass_kernel_spmd(
        nc, in_maps, core_ids=list(range(NCORES)))
    LAST_RESULTS = res
    rows = [np.concatenate([res.results[g * N_SPLIT + h]["outT"].T
                            for h in range(N_SPLIT)], axis=1)
            for g in range(M_GROUPS)]
    out = np.vstack(rows) if M_GROUPS > 1 else rows[0]
    return np.ascontiguousarray(out).reshape(B, S, D_OUT).astype(np.float32)


# revision 19
# speedup vs baseline: 1.6793x; 1.6793x over previous
"""BitLinear (ternary absmean-quantized linear) on 8 TRN2 NeuronCores.

Reference math (fp32):
    gamma = mean(|W|)
    Wq    = round(clip(W / (gamma + 1e-5), -1, 1))   # ternary {-1, 0, 1}
    out   = einsum('bsi,oi->bso', x, Wq)             # x @ Wq.T

Sharding: data-parallel over tokens. x [4,2048,4096] -> 8192 tokens, each
core owns 1024 of them and computes its full [1024, 4096] output slab with
no output collective. Every core needs the full quantized W; gamma (a global
scalar) is computed cooperatively: each core abs-sums 1/8 of W (512 of the
4096 output rows), a tiny [128,1] AllReduce combines the partials, and each
core then quantizes the full W on the fly while the TensorEngine consumes it.

Precision: hybrid bf16 / fp8-DoubleRow along K. Ternary weights are exact in
both bf16 and e4m3. x is exact enough in bf16 (rel err ~2e-3) and marginal in
e4m3 alone (~2.7e-2 > the 2e-2 gate), so the first KB K-planes run as bf16
matmuls and the remaining K-KB planes run as fp8 e4m3 matmuls with
perf_mode=DoubleRow (2 fp8 weights per PE cell, ~1.5-2x bf16 throughput).
Total rel err ~ 2.66e-2 * sqrt((K-KB)/K) ~= 1.8e-2 at KB=2176.

Device kernel layout (per core), output TRANSPOSED (features x tokens):
    xT   [4096, 1024] bf16  - this core's x slab, K-major (moving operand)
    WT   [4096, 4096] f32   - full W, transposed (in_features major)
    Wg   [4096,  512] f32   - this core's gamma shard (= 512 columns of WT)
    outT [4096, 1024] f32   - out.T; host transposes back

Main loop: 8 N-chunks of 512 output features. Per chunk: stream 32 K-slabs
of WT, quantize each on DVE ((|w| > t) with sign, 2 ops) into a resident
bf16 [128, KTB, 512] + fp8 [128, KTF, 512] chunk. Matmuls run with the
quantized W slab as the STATIONARY operand (one LDWEIGHTS per 2 matmuls of
512 tokens) and x as the moving operand, so weight loads amortize and the
PE streams 512-wide: per n-tile of 128 features, KTB bf16 matmuls + KTF/2
DoubleRow matmuls accumulate the full-K dot product in one PSUM bank.
"""

import numpy as np
import ml_dtypes

NCORES = 8

# Full-problem dims (hardcoded per the harness contract).
B, S, D_IN, D_OUT = 4, 2048, 4096, 4096
M_TOTAL = B * S            # 8192 tokens
N_SPLIT = 2                # n-halves (1: 8 token groups; 2: 4 groups x 2)
M_GROUPS = NCORES // N_SPLIT
M_CORE = M_TOTAL // M_GROUPS   # tokens per core
N_CORE = D_OUT // N_SPLIT      # output features per core
KB_PLANES = 2048           # K-planes computed in bf16 (rest fp8 DoubleRow)

_COMPILED = None   # cached (nc, meta)
LAST_RESULTS = None  # BassKernelResults of the most recent run (for test.py)


def build_module(m_core=M_CORE, k=D_IN, n=N_CORE, ncores=NCORES, repeat=1,
                 use_collective=True, kb=KB_PLANES, n_full=None,
                 wpool_bufs=None, stg_bufs=None, g_chunk=None):
    if wpool_bufs is None:
        wpool_bufs = 12 if m_core <= 1024 else 6
    if stg_bufs is None:
        stg_bufs = 4 if m_core <= 1024 else 2
    if g_chunk is None:
        g_chunk = 4 if m_core <= 1024 else 2
    # SBUF squeeze for the large-m (4x2-sharded) config
    opool_bufs = 6 if m_core <= 1024 else 4
    spool_bufs = 6 if m_core <= 1024 else 4
    """Build + compile the SPMD Bass module. Parametrized so a shrunken
    config can be validated in CoreSim. repeat>1 unrolls the whole kernel
    body multiple times inside one NEFF (for steady-state timing).

    n is this core's output-feature count; n_full (defaults to n) is the
    FULL problem's out_features, used only for the gamma normalization
    (gamma = sum|W| / (k * n_full), reduced across cores)."""
    import concourse.bass as bass  # noqa: F401
    import concourse.mybir as mybir
    import concourse.tile as tile
    from concourse import bacc
    from concourse import bass_isa

    f32 = mybir.dt.float32
    bf16 = mybir.dt.bfloat16
    f8 = mybir.dt.float8e4
    KT = k // 128            # total k-tiles of 128
    KTB = kb // 128          # bf16 k-tiles
    KTF = KT - KTB           # fp8 k-tiles
    KPAIRS = KTF // 2        # fp8 DoubleRow pairs
    KODD = KTF - 2 * KPAIRS  # leftover unpaired fp8 k-tile (0 or 1)
    NCHUNK = min(512, n)     # output-feature chunk width
    NCHUNKS = n // NCHUNK
    NTILES = NCHUNK // 128   # n-tiles (stationary free dim) per chunk
    MW = min(512, m_core)    # moving-operand token width
    MH = m_core // MW        # moving tiles per k-slab
    if n_full is None:
        n_full = n
    NG = n_full // 8         # gamma shard width (columns of full WT)
    G_CHUNK = min(g_chunk, KT)  # k-tiles per gamma reduce chunk
    G_CHUNKS = KT // G_CHUNK
    N_ELEMS = float(k * n_full)

    nc = bacc.Bacc("TRN2", target_bir_lowering=False, debug=False,
                   num_devices=ncores)
    xT = nc.dram_tensor("xT", [k, m_core], bf16, kind="ExternalInput")
    WT = nc.dram_tensor("WT", [k, n], f32, kind="ExternalInput")
    Wg = nc.dram_tensor("Wg", [k, NG], f32, kind="ExternalInput")
    outT = nc.dram_tensor("outT", [n, m_core], f32, kind="ExternalOutput")

    ts = bass.ts
    DR = mybir.MatmulPerfMode.DoubleRow

    with tile.TileContext(nc) as tc:
        with (
            tc.tile_pool(name="xpool", bufs=1) as xpool,
            tc.tile_pool(name="xstg", bufs=stg_bufs) as xstg,
            tc.tile_pool(name="gpool", bufs=2) as gpool,
            tc.tile_pool(name="wqbp", bufs=2) as wqbp,
            tc.tile_pool(name="wqfp", bufs=2) as wqfp,
            tc.tile_pool(name="wpool", bufs=wpool_bufs) as wpool,
            tc.tile_pool(name="spool", bufs=spool_bufs) as spool,
            tc.tile_pool(name="opool", bufs=opool_bufs) as opool,
            tc.tile_pool(name="small", bufs=2) as small,
            tc.tile_pool(name="pmain", bufs=8, space="PSUM") as pmain,
            tc.tile_pool(name="dram", bufs=2, space="DRAM") as dram,
        ):
          with tc.tile_pool(name="cpool", bufs=1) as cpool:
            bias_p = cpool.tile([128, 1], f32, name="bias_p")
            nc.gpsimd.memset(bias_p[:], 0.5e-5)
            bias_n = cpool.tile([128, 1], f32, name="bias_n")
            nc.gpsimd.memset(bias_n[:], -0.5e-5)

          # ---- resident x: bf16 [128, KTB, m] + fp8 [128, KTF, m] ----
          # Loaded once per NEFF execution (shared across repeat iterations;
          # x does not change within one launch). fp8 planes stage through
          # bf16 and cast on DVE.
          xb = xpool.tile([128, KTB, m_core], bf16, name="xb")
          xf = None
          if KTF:
              xf = xpool.tile([128, KTF, m_core], f8, name="xf")
          xr = xT[:, :].rearrange("(t p) m -> p t m", p=128)
          for kt in range(KT):
              if kt < KTB:
                  nc.sync.dma_start(xb[:, kt, :], xr[:, kt, :])
              else:
                  stg = xstg.tile([128, m_core], bf16, tag="xstg")
                  nc.sync.dma_start(stg[:], xr[:, kt, :])
                  nc.vector.tensor_copy(xf[:, kt - KTB, :], stg[:])

          for _rep in range(repeat):
            # ---- gamma: local abs-sum over this core's shard ----
            # Entirely on ACT + gpsimd (with its DMAs issued from the ACT
            # sequencer): these queues are idle during the main loop, so in
            # the repeat/steady-state case iteration i+1's whole gamma chain
            # (including the AllReduce) overlaps iteration i's matmuls
            # instead of queuing behind i's DVE/sync FIFOs.
            acc = small.tile([128, G_CHUNKS], f32)
            for j in range(G_CHUNKS):
                gsl = gpool.tile([128, G_CHUNK, NG], f32, tag="gsl")
                src = Wg[j * G_CHUNK * 128:(j + 1) * G_CHUNK * 128, :]
                # rep 0: sync queue -> gamma DMAs get strict head priority.
                # reps >0: ACT queue -> next iteration's gamma prefetch runs
                # under the current iteration's matmuls (sync FIFO is busy).
                geng = nc.sync if _rep == 0 else nc.scalar
                geng.dma_start(gsl[:], src.rearrange("(t p) c -> p t c", p=128))
                gscr = gpool.tile([128, G_CHUNK, NG], bf16, tag="gscr")
                nc.scalar.activation(
                    gscr[:], gsl[:], mybir.ActivationFunctionType.Abs,
                    accum_out=acc[:, j:j + 1])
            gpart = small.tile([128, 1], f32)
            gscr2 = small.tile([128, G_CHUNKS], bf16)
            nc.scalar.activation(
                gscr2[:], acc[:], mybir.ActivationFunctionType.Abs,
                accum_out=gpart[:])

            # ---- tiny AllReduce of per-partition partials ----
            gsum = small.tile([128, 1], f32)
            if ncores > 1 and use_collective:
                cin = dram.tile([128, 1], f32)
                nc.scalar.dma_start(cin[:], gpart[:])
                cout = dram.tile([128, 1], f32, tag="cout", name=f"cout{_rep}")
                nc.gpsimd.collective_compute(
                    "AllReduce", mybir.AluOpType.add,
                    replica_groups=[list(range(ncores))],
                    ins=[cin[:].opt()], outs=[cout[:].opt()])
                nc.scalar.dma_start(gsum[:], cout[:])
            else:
                # timing/TimelineSim variant: no collective (gamma from the
                # local shard only -- numerically wrong, timing-equivalent)
                nc.scalar.copy(gsum[:], gpart[:])

            # sum across partitions, result broadcast to all partitions
            gtot = small.tile([128, 1], f32)
            nc.gpsimd.partition_all_reduce(
                gtot[:], gsum[:], channels=128, reduce_op=bass_isa.ReduceOp.add)

            # threshold t = 0.5 * (gamma + 1e-5)
            # Wq = (w > t) - (w < -t)  in {-1, 0, 1}
            tsb = small.tile([128, 1], f32)
            nc.scalar.activation(
                tsb[:], gtot[:], mybir.ActivationFunctionType.Identity,
                bias=bias_p[:], scale=0.5 / N_ELEMS)
            ntsb = small.tile([128, 1], f32)
            nc.scalar.activation(
                ntsb[:], gtot[:], mybir.ActivationFunctionType.Identity,
                bias=bias_n[:], scale=-0.5 / N_ELEMS)

            # ---- main loop over output-feature chunks ----
            for c in range(NCHUNKS):
                # quantize this chunk's W: stream 32 k-slabs, 2 DVE ops each
                wqb = wqbp.tile([128, KTB, NCHUNK], bf16, tag="wqb")
                wqf = None
                if KTF:
                    wqf = wqfp.tile([128, KTF, NCHUNK], f8, tag="wqf",
                                    name="wqf")
                for kt in range(KT):
                    wtmp = wpool.tile([128, NCHUNK], f32, tag="wtmp")
                    nc.sync.dma_start(
                        wtmp[:], WT[ts(kt, 128), ts(c, NCHUNK)])
                    neg = spool.tile([128, NCHUNK], bf16, tag="neg")
                    nc.vector.tensor_scalar(
                        neg[:], wtmp[:], ntsb[:], None, mybir.AluOpType.is_lt)
                    dst = (wqb[:, kt, :] if kt < KTB
                           else wqf[:, kt - KTB, :])
                    nc.vector.scalar_tensor_tensor(
                        dst, wtmp[:], tsb[:], neg[:],
                        mybir.AluOpType.is_gt, mybir.AluOpType.subtract)

                # matmuls: stationary = wq n-tile (128 features), moving = x
                # (MW tokens). One PSUM bank accumulates full K per (nt, mh).
                for nt in range(NTILES):
                    ps = [pmain.tile([128, MW], f32, tag="ps",
                                     name=f"ps{nt % 2}_{mh}")
                          for mh in range(MH)]
                    n0 = nt * 128
                    for kt in range(KTB):
                        lw = wqb[:, kt, n0:n0 + 128]
                        for mh in range(MH):
                            nc.tensor.matmul(
                                ps[mh][:], lw, xb[:, kt, ts(mh, MW)],
                                start=(kt == 0), stop=False)
                    for j in range(KPAIRS):
                        lw = wqf[:, 2 * j:2 * j + 2, n0:n0 + 128]
                        last = (j == KPAIRS - 1) and KODD == 0
                        for mh in range(MH):
                            nc.tensor.matmul(
                                ps[mh][:], lw,
                                xf[:, 2 * j:2 * j + 2, ts(mh, MW)],
                                start=False, stop=last, perf_mode=DR)
                    if KODD:
                        lw = wqf[:, KTF - 1, n0:n0 + 128]
                        for mh in range(MH):
                            nc.tensor.matmul(
                                ps[mh][:], lw, xf[:, KTF - 1, ts(mh, MW)],
                                start=False, stop=True)
                    for mh in range(MH):
                        osb = opool.tile([128, MW], f32, tag="osb")
                        nc.vector.tensor_copy(osb[:], ps[mh][:])
                        nc.sync.dma_start(
                            outT[c * NCHUNK + n0:c * NCHUNK + n0 + 128,
                                 ts(mh, MW)], osb[:])

    nc.compile()
    meta = dict(m_core=m_core, k=k, n=n, ncores=ncores, NG=NG, kb=kb)
    return nc, meta


def _get_compiled():
    global _COMPILED
    if _COMPILED is None:
        _COMPILED = build_module(n_full=D_OUT)
    return _COMPILED


def make_in_maps(x, W, m_core=M_CORE, n_core=N_CORE, ncores=NCORES,
                 n_split=N_SPLIT):
    """Host-side shard prep. x [B,S,D_IN] f32, W [D_OUT,D_IN] f32.
    Core c = (token-group c//n_split, n-half c%n_split)."""
    k = W.shape[1]
    n = W.shape[0]
    ng = n // ncores
    x2 = np.asarray(x, dtype=np.float32).reshape(-1, k)
    xb = x2.astype(ml_dtypes.bfloat16)
    WT = np.ascontiguousarray(np.asarray(W, dtype=np.float32).T)  # [k, n]
    xTg = [np.ascontiguousarray(xb[g * m_core:(g + 1) * m_core, :].T)
           for g in range(ncores // n_split)]
    in_maps = []
    for c in range(ncores):
        g, h = divmod(c, n_split)
        WTc = (WT if n_split == 1 else
               np.ascontiguousarray(WT[:, h * n_core:(h + 1) * n_core]))
        Wgc = np.ascontiguousarray(WT[:, c * ng:(c + 1) * ng])
        in_maps.append({"xT": xTg[g], "WT": WTc, "Wg": Wgc})
    return in_maps


def kernel(input, W):
    """Full inputs in, full output out. Shards internally across 8 cores."""
    global LAST_RESULTS
    from concourse import bass_utils

    nc, meta = _get_compiled()
    in_maps = make_in_maps(input, W)
    res = bass_utils.run_bass_kernel_spmd(
        nc, in_maps, core_ids=list(range(NCORES)))
    LAST_RESULTS = res
    rows = [np.concatenate([res.results[g * N_SPLIT + h]["outT"].T
                            for h in range(N_SPLIT)], axis=1)
            for g in range(M_GROUPS)]
    out = np.vstack(rows) if M_GROUPS > 1 else rows[0]
    return np.ascontiguousarray(out).reshape(B, S, D_OUT).astype(np.float32)
